# revision 26
# baseline (speedup 1.0000x reference)
"""MoE ConditionalFeedForward (SwiGLU, T=2048 D=1024 I=4096 E=8 K=2) on 8 TRN2 cores.

Strategy: intermediate-slice parallel ("I-slice-8"). Every core handles ALL 8
experts, but only a 512-row slice (I/8) of each expert's w1/w3 rows and the
matching w2 columns. Per-core compute is then proportional to the TOTAL
routed-token count (perfectly balanced across cores by construction), not to
the max expert load, and each expert's matmul free dim is its exact (rounded
to 8) token count - no padding to a shared CAP. Layer-2 outputs are partial
sums over the I-slice; the host adds the 8 per-core partials during the
gather. Gate scaling stays on-device (per-partition scalar at the drain), so
sum_c g*y_c == g*y.

Device kernel (per core):
  Experts are processed as blocks: L1(e) = 4 i-tiles x 16 matmuls (N=ncol_e)
  into h1/h3 PSUM, silu+mul into a resident hT [128, 4, sum(ncol)] bf16 tile;
  L2(e-1) = 2 d-halves x 4 token-tiles x 4-matmul accumulation chains
  (N=512) is emitted between L1 blocks so its hT dependency is ~13us stale
  (no PE stall). Drains: ACT(Copy*gate) low half + DVE(tensor_scalar_mul)
  high half into bf16 y_sb, then one DMA per (dc,tt) on a rotating ring.

  DMA: only gpsimd/sync/scalar rings exist; a ring's entries run in FIFO
  order, so queue position paces transfers. Ramp: xt(e0) split sync+scalar,
  w13 head on gpsimd, then 3-way rotation; w2/xt(e+1) triggers are issued a
  full expert-block early so they sit deep enough in the FIFO not to steal
  ramp bandwidth but land before use. All packed layouts are partition-major
  so every DMA is a rearrange-free linear read (256 B-chunk reads halve SDMA
  efficiency).

  The profiler's measured window opens at framework init memsets (~6.5us);
  ~100 warmup matmuls (~56ns each) keep the PE busy (HAM clock-gate 8/8)
  until the ~350 GB/s HBM stream has delivered xt(e0)+w13(e0,it0).
"""

import math
import os
import sys
import time
import types

for _p in ("/opt/trn_rl_repo", "/opt/pypackages"):
    if _p not in sys.path:
        sys.path.append(_p)

import numpy as np
import ml_dtypes

# antenv.axon_hooks is absent from this image; run_bass_kernel_spmd imports it
# unconditionally when tracing is requested (BASS_TRACE=1). Provide the
# documented shim so profiling works when asked for and degrades to a no-op
# otherwise. No-op if a real antenv.axon_hooks exists.
def _ensure_ntff_hook():
    try:
        import antenv
    except ImportError:
        return
    try:
        import antenv.axon_hooks  # noqa: F401
        return
    except ImportError:
        pass
    mod = types.ModuleType("antenv.axon_hooks")
    mod._hook = None

    def set_axon_ntff_profile_hook(h):
        mod._hook = h

    def get_axon_ntff_profile_hook():
        if mod._hook is None:
            try:
                from trn_agent_boot.trn_boot import _ntff_profile_via_ctypes

                mod._hook = _ntff_profile_via_ctypes("/opt/axon/libaxon_pjrt.so")
            except Exception:
                mod._hook = None
        return mod._hook

    mod.set_axon_ntff_profile_hook = set_axon_ntff_profile_hook
    mod.get_axon_ntff_profile_hook = get_axon_ntff_profile_hook
    sys.modules["antenv.axon_hooks"] = mod
    antenv.axon_hooks = mod


_ensure_ntff_hook()

import concourse.bacc as bacc
import concourse.tile as tile
from concourse import mybir
from concourse.bass_utils import run_bass_kernel_spmd

T, D, I, E, TOPK = 2048, 1024, 4096, 8, 2
N_CORES = 8
IS = I // N_CORES    # 512 intermediate rows per expert per core
NIT = IS // 128      # 4 i-tiles per expert
DT = D // 128        # 8 contraction steps for layer 1
CAP = 512            # max tokens per expert per pass (PSUM free-dim limit)
F32 = mybir.dt.float32
BF16 = mybir.dt.bfloat16
BF16_NP = ml_dtypes.bfloat16
N_WARM = 128         # tiny matmuls (~56ns each) spanning the DMA ramp

_NCS = {}            # compiled Bass modules, keyed on the ncols tuple
_WCACHE = {}         # packed per-core weights, keyed on input identity
LAST_RESULTS = None  # BassKernelResults of the most recent SPMD run


def _build_nc(ncols, sim_act=False):
    # sim_act: CoreSim lacks Silu; emit sigmoid + extra multiply instead
    # (same math) so the program can be validated in simulation.
    ncols = tuple(ncols)
    off = [0]
    for n in ncols:
        off.append(off[-1] + n)
    SN = off[-1]
    nts = [(n + 127) // 128 for n in ncols]

    nc = bacc.Bacc(
        "TRN2", target_bir_lowering=False, debug=False, num_devices=N_CORES
    )
    xt_d = nc.dram_tensor("xt", [128, DT, SN], BF16, kind="ExternalInput").ap()
    g_d = nc.dram_tensor("g", [128, 4 * E], F32, kind="ExternalInput").ap()
    w13p_d = nc.dram_tensor(
        "w13p", [E, NIT, 128, 2, DT, 128], BF16, kind="ExternalInput"
    ).ap()
    w2p_d = nc.dram_tensor(
        "w2p", [E, 2, 128, NIT, 512], BF16, kind="ExternalInput"
    ).ap()
    y_d = nc.dram_tensor("y", [SN, D], BF16, kind="ExternalOutput").ap()

    with tile.TileContext(nc) as tc:
        with (
            tc.tile_pool(name="consts", bufs=1) as const_pool,
            tc.tile_pool(name="w13", bufs=9) as w13_pool,
            tc.tile_pool(name="w2", bufs=6) as w2_pool,
            tc.tile_pool(name="h", bufs=1) as h_pool,
            tc.tile_pool(name="tmp", bufs=2) as tmp_pool,
            tc.tile_pool(name="yout", bufs=4) as out_pool,
        ):
            warm_sb = const_pool.tile([128, 64], BF16)
            nc.vector.memset(warm_sb[:], 0.0)
            warm_pool = tc.alloc_tile_pool(name="warm", bufs=1, space="PSUM")
            warm_ps = warm_pool.tile([64, 64], F32)
            for _ in range(N_WARM):
                nc.tensor.matmul(
                    warm_ps[:], warm_sb[:, :64], warm_sb[:, :64],
                    start=True, stop=True,
                )
            warm_pool.release()

            # Resident tensors: all experts' gathered x^T and layer-1 output.
            xt_sb = const_pool.tile([128, DT, SN], BF16)
            g_sb = const_pool.tile([128, 4 * E], F32)
            hT = h_pool.tile([128, NIT, SN], BF16)

            # xt arrives per-expert in consumption order, split sync/scalar.
            def load_xt(e):
                o0, o1 = off[e], off[e + 1]
                mid = o0 + (((o1 - o0) // 2) + 7) // 8 * 8
                nc.sync.dma_start(xt_sb[:, :, o0:mid], xt_d[:, :, o0:mid])
                nc.scalar.dma_start(xt_sb[:, :, mid:o1], xt_d[:, :, mid:o1])

            load_xt(0)

            ps2_pool = tc.alloc_tile_pool(name="ps2", bufs=1, space="PSUM")
            ps1_pool = tc.alloc_tile_pool(name="ps1", bufs=2, space="PSUM")

            w13_rings = [nc.gpsimd, nc.sync, nc.scalar]
            w2_tiles = {}
            drain_ring = [nc.gpsimd, nc.sync, nc.scalar]

            def emit_l1(e):
                ncol = ncols[e]
                for itl in range(NIT):
                    k = e * NIT + itl
                    w13_t = w13_pool.tile([128, 2, DT, 128], BF16, tag="w13")
                    # The gpsimd (software-DGE) queue has ~3.4us trigger-to-
                    # first-byte latency, so the ramp-critical head (it0/it1,
                    # behind the xt halves) rides the two HW-DGE queues;
                    # gpsimd opens with it2/it3; then a 3-way rotation keeps
                    # every ring under its ~1 tile/2.9us receipt rate.
                    if k == 0:
                        # First tile split by matrix across the two HW
                        # queues: the h1 chain only needs the w1 half, so
                        # the first real matmul is gated on xt + 256 KB.
                        nc.sync.dma_start(w13_t[:, 0], w13p_d[e, itl, :, 0])
                        nc.scalar.dma_start(w13_t[:, 1], w13p_d[e, itl, :, 1])
                    else:
                        if k == 1:
                            eng = nc.scalar
                        elif k in (2, 3):
                            eng = nc.gpsimd
                        else:
                            eng = w13_rings[k % 3]
                        eng.dma_start(w13_t[:], w13p_d[e, itl])
                    if k == 1:
                        # Gates: tiny, needed first at L2(e0)'s drains.
                        nc.gpsimd.dma_start(g_sb[:], g_d[:, :])
                    if itl == 1:
                        # w2(e) quads, issued one i-tile into L1(e): deep
                        # enough in the FIFO not to race the w13 stream,
                        # landed before L2(e) starts (after L1(e+1)).
                        for dc in range(2):
                            w2_t = w2_pool.tile(
                                [128, NIT, 512], BF16, tag="w2"
                            )
                            (nc.gpsimd if dc == 0 else nc.sync).dma_start(
                                w2_t[:], w2p_d[e, dc]
                            )
                            w2_tiles[(e, dc)] = w2_t
                    h1_ps = ps1_pool.tile([128, ncol], F32, tag="h1")
                    h3_ps = ps1_pool.tile([128, ncol], F32, tag="h3")
                    for dt_i in range(DT):
                        nc.tensor.matmul(
                            h1_ps[:],
                            w13_t[:, 0, dt_i, :],
                            xt_sb[:, dt_i, off[e]:off[e] + ncol],
                            start=(dt_i == 0),
                            stop=(dt_i == DT - 1),
                        )
                    for dt_i in range(DT):
                        nc.tensor.matmul(
                            h3_ps[:],
                            w13_t[:, 1, dt_i, :],
                            xt_sb[:, dt_i, off[e]:off[e] + ncol],
                            start=(dt_i == 0),
                            stop=(dt_i == DT - 1),
                        )
                    s_sb = tmp_pool.tile([128, ncol], F32)
                    if sim_act:
                        nc.scalar.activation(
                            s_sb[:], h1_ps[:],
                            mybir.ActivationFunctionType.Sigmoid,
                        )
                        nc.vector.tensor_mul(s_sb[:], s_sb[:], h1_ps[:])
                    else:
                        nc.scalar.activation(
                            s_sb[:], h1_ps[:],
                            mybir.ActivationFunctionType.Silu,
                        )
                    nc.vector.tensor_mul(
                        hT[:, itl, off[e]:off[e] + ncol], s_sb[:], h3_ps[:]
                    )

            def emit_l2(e, last=False):
                ncol, nt = ncols[e], nts[e]

                def rows(tt):
                    return min(128, ncol - 128 * tt)

                for dc in range(2):
                    w2_t = w2_tiles.pop((e, dc))
                    y_ps = [
                        ps2_pool.tile(
                            [128, 512], F32, tag=f"y{tt}", name=f"y{e}_{dc}_{tt}"
                        )
                        for tt in range(nt)
                    ]

                    def l2mm(tt, itl):
                        r = rows(tt)
                        o = off[e] + tt * 128
                        nc.tensor.matmul(
                            y_ps[tt][:r, :],
                            hT[:, itl, o:o + r],
                            w2_t[:, itl, :],
                            start=(itl == 0),
                            stop=(itl == NIT - 1),
                        )

                    for itl in range(NIT - 1):
                        for tt in range(nt):
                            l2mm(tt, itl)
                    for tt in range(nt):
                        l2mm(tt, NIT - 1)
                        # Drain tt right after its last matmul.
                        r = rows(tt)
                        y_sb = out_pool.tile([128, 512], BF16)
                        src = y_ps[tt]
                        gs = g_sb[:r, 4 * e + tt:4 * e + tt + 1]
                        lo, hi = (slice(0, 256), slice(256, 512))
                        if tt % 2:
                            lo, hi = hi, lo
                        nc.scalar.activation(
                            y_sb[:r, lo], src[:r, lo],
                            mybir.ActivationFunctionType.Copy, scale=gs,
                        )
                        nc.vector.tensor_scalar_mul(y_sb[:r, hi], src[:r, hi], gs)
                        dst = y_d[
                            off[e] + tt * 128:off[e] + tt * 128 + r,
                            dc * 512:(dc + 1) * 512,
                        ]
                        if last and dc == 1:
                            # Kernel tail: HW-DGE rings only — gpsimd's
                            # software queue takes ~3.6us to drain its
                            # completion receipts and would gate the exit
                            # barrier.
                            eng = (nc.sync, nc.scalar, nc.scalar, nc.sync)[tt % 4]
                        else:
                            eng = drain_ring[(e * 2 + dc + tt) % 3]
                        eng.dma_start(dst, y_sb[:r, :])

            # Expert blocks, L2 one block behind L1 so its hT dependency is
            # ~13us stale by the time the PE reaches it.
            for e in range(E):
                emit_l1(e)
                if e == 0 and E > 1:
                    load_xt(1)
                if e + 2 < E:
                    load_xt(e + 2)
                if e >= 1:
                    emit_l2(e - 1)
            emit_l2(E - 1, last=True)

            ps1_pool.release()
            ps2_pool.release()

    nc.compile()
    return nc


def _pack_weights(w1, w2, w3):
    """Per-core device layouts (bf16), partition-major so every DMA is a
    rearrange-free linear read:
    w13p[e, itl, p, m, dt, c] = wm[e][cslice + itl*128 + c, dt*128 + p]
    w2p[e, dc, p, a, n] = w2[e][dc*512 + n, cslice + a*128 + p]."""
    key = tuple((a.ctypes.data, a.shape) for a in (w1, w2, w3))
    if _WCACHE.get("key") == key:
        return _WCACHE["maps"]
    maps = []
    w1r = w1.reshape(E, E * NIT, 128, DT, 128)
    w3r = w3.reshape(E, E * NIT, 128, DT, 128)
    w2r = np.ascontiguousarray(np.transpose(w2, (0, 2, 1))).reshape(
        E, E * NIT, 128, 2, 512
    )
    for c in range(N_CORES):
        sl = slice(c * NIT, c * NIT + NIT)
        w13p = np.empty((E, NIT, 128, 2, DT, 128), dtype=BF16_NP)
        w13p[:, :, :, 0] = w1r[:, sl].transpose(0, 1, 4, 3, 2)
        w13p[:, :, :, 1] = w3r[:, sl].transpose(0, 1, 4, 3, 2)
        w2p = np.ascontiguousarray(
            w2r[:, sl].transpose(0, 3, 2, 1, 4)
        ).astype(BF16_NP)
        maps.append({"w13p": w13p, "w2p": w2p})
    _WCACHE["key"] = key
    _WCACHE["maps"] = maps
    return maps


def kernel(x, expert_indices, expert_weights, w1, w2, w3):
    global LAST_RESULTS
    x = np.ascontiguousarray(np.asarray(x, dtype=np.float32))
    idx = np.asarray(expert_indices)
    ew = np.asarray(expert_weights, dtype=np.float32)
    w1 = np.ascontiguousarray(np.asarray(w1, dtype=np.float32))
    w2 = np.ascontiguousarray(np.asarray(w2, dtype=np.float32))
    w3 = np.ascontiguousarray(np.asarray(w3, dtype=np.float32))

    # Host routing: unique tokens per expert, with both top-k gate weights of
    # a token merged (a token picking the same expert twice gets the summed
    # gate).
    tok_lists, gate_lists = [], []
    for e in range(E):
        m = idx == e
        sel = np.nonzero(m.any(axis=1))[0]
        tok_lists.append(sel)
        gate_lists.append((ew * m).sum(axis=1)[sel].astype(np.float32))

    weight_maps = _pack_weights(w1, w2, w3)

    n_pass = max(1, math.ceil(max(len(s) for s in tok_lists) / CAP))
    out = np.zeros((T, D), dtype=np.float32)
    trace = bool(os.environ.get("BASS_TRACE"))
    for p in range(n_pass):
        chunks = [s[p * CAP:(p + 1) * CAP] for s in tok_lists]
        gates = [g[p * CAP:(p + 1) * CAP] for g in gate_lists]
        ncols = tuple(max(8, -(-len(s) // 8) * 8) for s in chunks)
        if ncols not in _NCS:
            _NCS[ncols] = _build_nc(ncols)
        nc_mod = _NCS[ncols]

        off = [0]
        for n in ncols:
            off.append(off[-1] + n)
        SN = off[-1]
        xt = np.zeros((128, DT, SN), dtype=BF16_NP)
        g_pad = np.zeros((128, 4 * E), dtype=np.float32)
        for e in range(E):
            sel = chunks[e]
            if len(sel):
                xe = np.zeros((D, ncols[e]), dtype=np.float32)
                xe[:, :len(sel)] = x[sel].T
                xt[:, :, off[e]:off[e + 1]] = (
                    xe.astype(BF16_NP).reshape(DT, 128, ncols[e]).transpose(1, 0, 2)
                )
                gp = np.zeros((512,), dtype=np.float32)
                gp[:len(sel)] = gates[e]
                g_pad[:, 4 * e:4 * e + 4] = gp.reshape(4, 128).T
        in_maps = [
            {"xt": xt, "g": g_pad, **weight_maps[c]} for c in range(N_CORES)
        ]
        # Rare transient NRT_EXEC_UNIT_UNRECOVERABLE errors have been
        # observed on the first execution of a fresh NEFF; a straight retry
        # recovers.
        last_exc = None
        for attempt in range(3):
            try:
                LAST_RESULTS = run_bass_kernel_spmd(
                    nc_mod, in_maps, core_ids=list(range(N_CORES)),
                    trace=trace and attempt == 0,
                )
                break
            except Exception as exc:  # noqa: BLE001
                last_exc = exc
                time.sleep(3)
        else:
            raise last_exc
        for c in range(N_CORES):
            yc = LAST_RESULTS.results[c]["y"]
            for e in range(E):
                sel = chunks[e]
                if len(sel):
                    out[sel] += yc[off[e]:off[e] + len(sel)].astype(np.float32)
    return out


# revision 29
# speedup vs baseline: 1.0113x; 1.0113x over previous
"""MoE ConditionalFeedForward (SwiGLU, T=2048 D=1024 I=4096 E=8 K=2) on 8 TRN2 cores.

Strategy: intermediate-slice parallel ("I-slice-8"). Every core handles ALL 8
experts, but only a 512-row slice (I/8) of each expert's w1/w3 rows and the
matching w2 columns. Per-core compute is then proportional to the TOTAL
routed-token count (perfectly balanced across cores by construction), not to
the max expert load, and each expert's matmul free dim is its exact (rounded
to 8) token count - no padding to a shared CAP. Layer-2 outputs are partial
sums over the I-slice; the host adds the 8 per-core partials during the
gather. Gate scaling stays on-device (per-partition scalar at the drain), so
sum_c g*y_c == g*y.

Device kernel (per core):
  Experts are processed as blocks: L1(e) = 4 i-tiles x 16 matmuls (N=ncol_e)
  into h1/h3 PSUM, silu+mul into a resident hT [128, 4, sum(ncol)] bf16 tile;
  L2(e-1) = 2 d-halves x 4 token-tiles x 4-matmul accumulation chains
  (N=512) is emitted between L1 blocks so its hT dependency is ~13us stale
  (no PE stall). Drains: ACT(Copy*gate) low half + DVE(tensor_scalar_mul)
  high half into bf16 y_sb, then one DMA per (dc,tt) on a rotating ring.

  DMA: only gpsimd/sync/scalar rings exist; a ring's entries run in FIFO
  order, so queue position paces transfers. Ramp: xt(e0) split sync+scalar,
  w13 head on gpsimd, then 3-way rotation; w2/xt(e+1) triggers are issued a
  full expert-block early so they sit deep enough in the FIFO not to steal
  ramp bandwidth but land before use. All packed layouts are partition-major
  so every DMA is a rearrange-free linear read (256 B-chunk reads halve SDMA
  efficiency).

  The profiler's measured window opens at framework init memsets (~6.5us);
  ~100 warmup matmuls (~56ns each) keep the PE busy (HAM clock-gate 8/8)
  until the ~350 GB/s HBM stream has delivered xt(e0)+w13(e0,it0).
"""

import math
import os
import sys
import time
import types

for _p in ("/opt/trn_rl_repo", "/opt/pypackages"):
    if _p not in sys.path:
        sys.path.append(_p)

import numpy as np
import ml_dtypes

# antenv.axon_hooks is absent from this image; run_bass_kernel_spmd imports it
# unconditionally when tracing is requested (BASS_TRACE=1). Provide the
# documented shim so profiling works when asked for and degrades to a no-op
# otherwise. No-op if a real antenv.axon_hooks exists.
def _ensure_ntff_hook():
    try:
        import antenv
    except ImportError:
        return
    try:
        import antenv.axon_hooks  # noqa: F401
        return
    except ImportError:
        pass
    mod = types.ModuleType("antenv.axon_hooks")
    mod._hook = None

    def set_axon_ntff_profile_hook(h):
        mod._hook = h

    def get_axon_ntff_profile_hook():
        if mod._hook is None:
            try:
                from trn_agent_boot.trn_boot import _ntff_profile_via_ctypes

                mod._hook = _ntff_profile_via_ctypes("/opt/axon/libaxon_pjrt.so")
            except Exception:
                mod._hook = None
        return mod._hook

    mod.set_axon_ntff_profile_hook = set_axon_ntff_profile_hook
    mod.get_axon_ntff_profile_hook = get_axon_ntff_profile_hook
    sys.modules["antenv.axon_hooks"] = mod
    antenv.axon_hooks = mod


_ensure_ntff_hook()

import concourse.bacc as bacc
import concourse.tile as tile
from concourse import mybir
from concourse.bass_utils import run_bass_kernel_spmd

T, D, I, E, TOPK = 2048, 1024, 4096, 8, 2
N_CORES = 8
IS = I // N_CORES    # 512 intermediate rows per expert per core
NIT = IS // 128      # 4 i-tiles per expert
DT = D // 128        # 8 contraction steps for layer 1
CAP = 512            # max tokens per expert per pass (PSUM free-dim limit)
F32 = mybir.dt.float32
BF16 = mybir.dt.bfloat16
BF16_NP = ml_dtypes.bfloat16
N_WARM = 128         # tiny matmuls (~56ns each) spanning the DMA ramp

_NCS = {}            # compiled Bass modules, keyed on the ncols tuple
_WCACHE = {}         # packed per-core weights, keyed on input identity
LAST_RESULTS = None  # BassKernelResults of the most recent SPMD run


def _build_nc(ncols, sim_act=False):
    # sim_act: CoreSim lacks Silu; emit sigmoid + extra multiply instead
    # (same math) so the program can be validated in simulation.
    ncols = tuple(ncols)
    off = [0]
    for n in ncols:
        off.append(off[-1] + n)
    SN = off[-1]
    nts = [(n + 127) // 128 for n in ncols]

    nc = bacc.Bacc(
        "TRN2", target_bir_lowering=False, debug=False, num_devices=N_CORES
    )
    xt_d = nc.dram_tensor("xt", [128, DT, SN], BF16, kind="ExternalInput").ap()
    g_d = nc.dram_tensor("g", [128, 4 * E], F32, kind="ExternalInput").ap()
    w13p_d = nc.dram_tensor(
        "w13p", [E, 128, NIT, 2, DT, 128], BF16, kind="ExternalInput"
    ).ap()
    w2p_d = nc.dram_tensor(
        "w2p", [E, 128, 2, NIT, 512], BF16, kind="ExternalInput"
    ).ap()
    y_d = nc.dram_tensor("y", [SN, D], BF16, kind="ExternalOutput").ap()

    with tile.TileContext(nc) as tc:
        with (
            tc.tile_pool(name="consts", bufs=1) as const_pool,
            tc.tile_pool(name="w13", bufs=3) as w13_pool,
            tc.tile_pool(name="w2", bufs=3) as w2_pool,
            tc.tile_pool(name="h", bufs=1) as h_pool,
            tc.tile_pool(name="tmp", bufs=2) as tmp_pool,
            tc.tile_pool(name="yout", bufs=2) as out_pool,
        ):
            # PE warmup: the profiler's measured window opens at framework
            # init memsets (~6.5us) no matter what we emit, and the first
            # real matmul is gated on the ~350 GB/s HBM stream delivering
            # xt(e0) + the first w1 half (~1.2 MB -> ready ~14us). Warmup
            # matmuls (~56ns each) keep the PE busy until then so the HAM
            # clock-gate is at 8/8 throughout.
            warm_sb = const_pool.tile([128, 64], BF16)
            nc.vector.memset(warm_sb[:], 0.0)
            warm_pool = tc.alloc_tile_pool(name="warm", bufs=1, space="PSUM")
            warm_ps = warm_pool.tile([64, 64], F32)
            for _ in range(N_WARM):
                nc.tensor.matmul(
                    warm_ps[:], warm_sb[:, :64], warm_sb[:, :64],
                    start=True, stop=True,
                )
            warm_pool.release()

            # Resident tensors: all experts' gathered x^T and layer-1 output.
            xt_sb = const_pool.tile([128, DT, SN], BF16)
            g_sb = const_pool.tile([128, 4 * E], F32)
            hT = h_pool.tile([128, NIT, SN], BF16)

            # DMA plan. Rings serialize transfer + ~1.4us completion receipt
            # per entry, so everything moves as few, big, rearrange-free
            # transfers: one 2 MB w13 + one 1 MB w2 + one ~1 MB xt + one
            # ~1 MB y writeout per expert block (~20us), spread over the
            # three rings with 1-2 blocks of prefetch lead. Exception: the
            # ramp-critical head (xt(e0) halves + w13(e0) pieces) is split
            # across the two fast HW-DGE queues (gpsimd's software queue has
            # ~3.4us trigger-to-first-byte latency) so the first real matmul
            # is gated on just xt + the 256 KB w1 half of it0.
            w13_tiles, w2_tiles = {}, {}

            def load_xt(e, split=False):
                o0, o1 = off[e], off[e + 1]
                if split:
                    mid = o0 + (((o1 - o0) // 2) + 7) // 8 * 8
                    nc.sync.dma_start(xt_sb[:, :, o0:mid], xt_d[:, :, o0:mid])
                    nc.scalar.dma_start(xt_sb[:, :, mid:o1], xt_d[:, :, mid:o1])
                else:
                    (nc.sync, nc.scalar)[e % 2].dma_start(
                        xt_sb[:, :, o0:o1], xt_d[:, :, o0:o1]
                    )

            def load_w13(e):
                w13_t = w13_pool.tile([128, NIT, 2, DT, 128], BF16, tag="w13e")
                if e == 0:
                    nc.sync.dma_start(w13_t[:, 0, 0], w13p_d[0, :, 0, 0])
                    nc.scalar.dma_start(w13_t[:, 0, 1], w13p_d[0, :, 0, 1])
                    nc.scalar.dma_start(w13_t[:, 1], w13p_d[0, :, 1])
                    nc.gpsimd.dma_start(w13_t[:, 2], w13p_d[0, :, 2])
                    nc.gpsimd.dma_start(w13_t[:, 3], w13p_d[0, :, 3])
                else:
                    (nc.gpsimd, nc.sync, nc.scalar)[e % 3].dma_start(
                        w13_t[:], w13p_d[e]
                    )
                w13_tiles[e] = w13_t

            def load_w2(e):
                w2_t = w2_pool.tile([128, 2, NIT, 512], BF16, tag="w2e")
                (nc.scalar, nc.gpsimd, nc.sync)[e % 3].dma_start(
                    w2_t[:], w2p_d[e]
                )
                w2_tiles[e] = w2_t

            ps2_pool = tc.alloc_tile_pool(name="ps2", bufs=1, space="PSUM")
            ps1_pool = tc.alloc_tile_pool(name="ps1", bufs=2, space="PSUM")

            def emit_l1(e):
                ncol = ncols[e]
                w13_t = w13_tiles.pop(e)
                for itl in range(NIT):
                    h1_ps = ps1_pool.tile([128, ncol], F32, tag="h1")
                    h3_ps = ps1_pool.tile([128, ncol], F32, tag="h3")
                    for dt_i in range(DT):
                        nc.tensor.matmul(
                            h1_ps[:],
                            w13_t[:, itl, 0, dt_i, :],
                            xt_sb[:, dt_i, off[e]:off[e] + ncol],
                            start=(dt_i == 0),
                            stop=(dt_i == DT - 1),
                        )
                    for dt_i in range(DT):
                        nc.tensor.matmul(
                            h3_ps[:],
                            w13_t[:, itl, 1, dt_i, :],
                            xt_sb[:, dt_i, off[e]:off[e] + ncol],
                            start=(dt_i == 0),
                            stop=(dt_i == DT - 1),
                        )
                    s_sb = tmp_pool.tile([128, ncol], F32)
                    if sim_act:
                        nc.scalar.activation(
                            s_sb[:], h1_ps[:],
                            mybir.ActivationFunctionType.Sigmoid,
                        )
                        nc.vector.tensor_mul(s_sb[:], s_sb[:], h1_ps[:])
                    else:
                        nc.scalar.activation(
                            s_sb[:], h1_ps[:],
                            mybir.ActivationFunctionType.Silu,
                        )
                    nc.vector.tensor_mul(
                        hT[:, itl, off[e]:off[e] + ncol], s_sb[:], h3_ps[:]
                    )

            def emit_l2(e, last=False):
                ncol, nt = ncols[e], nts[e]
                w2_t = w2_tiles.pop(e)
                y_sb = out_pool.tile([128, nts[e], 1024], BF16, tag="ye")

                def rows(tt):
                    return min(128, ncol - 128 * tt)

                for dc in range(2):
                    y_ps = [
                        ps2_pool.tile(
                            [128, 512], F32, tag=f"y{tt}", name=f"y{e}_{dc}_{tt}"
                        )
                        for tt in range(nt)
                    ]

                    def l2mm(tt, itl):
                        r = rows(tt)
                        o = off[e] + tt * 128
                        nc.tensor.matmul(
                            y_ps[tt][:r, :],
                            hT[:, itl, o:o + r],
                            w2_t[:, dc, itl, :],
                            start=(itl == 0),
                            stop=(itl == NIT - 1),
                        )

                    for itl in range(NIT - 1):
                        for tt in range(nt):
                            l2mm(tt, itl)
                    for tt in range(nt):
                        l2mm(tt, NIT - 1)
                        # Drain tt right after its last matmul: gate-scaled
                        # PSUM -> bf16 y_sb, split ACT (low half) / DVE
                        # (high half).
                        r = rows(tt)
                        src = y_ps[tt]
                        gs = g_sb[:r, 4 * e + tt:4 * e + tt + 1]
                        ls, hs = slice(0, 256), slice(256, 512)
                        if tt % 2:
                            ls, hs = hs, ls
                        nc.scalar.activation(
                            y_sb[:r, tt, dc * 512 + ls.start:dc * 512 + ls.stop],
                            src[:r, ls],
                            mybir.ActivationFunctionType.Copy, scale=gs,
                        )
                        nc.vector.tensor_scalar_mul(
                            y_sb[:r, tt, dc * 512 + hs.start:dc * 512 + hs.stop],
                            src[:r, hs], gs,
                        )
                        if last and dc == 1:
                            # Kernel tail: small per-tile DMAs on the two
                            # HW-DGE rings only (gpsimd takes ~3.6us to
                            # drain receipts and would gate the exit
                            # barrier); the very last rides sync.
                            eng = (nc.sync, nc.scalar, nc.scalar, nc.sync)[tt % 4]
                            o = off[e] + tt * 128
                            eng.dma_start(
                                y_d[o:o + r, 512:1024], y_sb[:r, tt, 512:1024]
                            )
                if last:
                    # dc0 of the final expert: batched writeout on gpsimd,
                    # ~7us before the exit barrier so its slow queue-drain
                    # clears in time.
                    _drain_ragged(nc.gpsimd, y_d, y_sb, off[e], ncol, nt, 0)
                else:
                    # Both halves of this expert's partials, two batched
                    # rearrange-free DMAs (full token-tiles + ragged last).
                    _drain_ragged(
                        (nc.gpsimd, nc.sync, nc.scalar)[e % 3],
                        y_d, y_sb, off[e], ncol, nt, None,
                    )

            def _drain_ragged(eng, y_dd, y_sb, o0, ncol, nt, dc):
                cs = slice(0, 1024) if dc is None else slice(dc * 512, (dc + 1) * 512)
                r_last = ncol - 128 * (nt - 1)
                if nt > 1:
                    eng.dma_start(
                        y_dd[o0:o0 + (nt - 1) * 128, cs].rearrange(
                            "(a p) c -> p a c", p=128
                        ),
                        y_sb[:, :nt - 1, cs],
                    )
                eng.dma_start(
                    y_dd[o0 + (nt - 1) * 128:o0 + ncol, cs],
                    y_sb[:r_last, nt - 1, cs],
                )

            # Prefetch head, then expert blocks with L2 one block behind L1
            # so its hT dependency is ~13us stale by the time the PE reaches
            # it (no stall on the silu/mul chain).
            load_xt(0, split=True)
            load_w13(0)
            load_w2(0)
            load_w13(1)
            load_xt(1)
            nc.gpsimd.dma_start(g_sb[:], g_d[:, :])
            for e in range(E):
                if e + 2 < E:
                    load_w13(e + 2)
                    load_xt(e + 2)
                if e + 1 < E:
                    load_w2(e + 1)
                emit_l1(e)
                if e >= 1:
                    emit_l2(e - 1)
            emit_l2(E - 1, last=True)

            ps1_pool.release()
            ps2_pool.release()

    nc.compile()
    return nc


def _pack_weights(w1, w2, w3):
    """Per-core device layouts (bf16), partition-major so every DMA is a
    rearrange-free linear read:
    w13p[e, p, itl, m, dt, c] = wm[e][cslice + itl*128 + c, dt*128 + p]
    w2p[e, p, dc, a, n] = w2[e][dc*512 + n, cslice + a*128 + p]."""
    key = tuple((a.ctypes.data, a.shape) for a in (w1, w2, w3))
    if _WCACHE.get("key") == key:
        return _WCACHE["maps"]
    maps = []
    w1r = w1.reshape(E, E * NIT, 128, DT, 128)
    w3r = w3.reshape(E, E * NIT, 128, DT, 128)
    w2r = np.ascontiguousarray(np.transpose(w2, (0, 2, 1))).reshape(
        E, E * NIT, 128, 2, 512
    )
    for c in range(N_CORES):
        sl = slice(c * NIT, c * NIT + NIT)
        w13p = np.empty((E, 128, NIT, 2, DT, 128), dtype=BF16_NP)
        w13p[:, :, :, 0] = w1r[:, sl].transpose(0, 4, 1, 3, 2)
        w13p[:, :, :, 1] = w3r[:, sl].transpose(0, 4, 1, 3, 2)
        w2p = np.ascontiguousarray(
            w2r[:, sl].transpose(0, 2, 3, 1, 4)
        ).astype(BF16_NP)
        maps.append({"w13p": w13p, "w2p": w2p})
    _WCACHE["key"] = key
    _WCACHE["maps"] = maps
    return maps


def kernel(x, expert_indices, expert_weights, w1, w2, w3):
    global LAST_RESULTS
    x = np.ascontiguousarray(np.asarray(x, dtype=np.float32))
    idx = np.asarray(expert_indices)
    ew = np.asarray(expert_weights, dtype=np.float32)
    w1 = np.ascontiguousarray(np.asarray(w1, dtype=np.float32))
    w2 = np.ascontiguousarray(np.asarray(w2, dtype=np.float32))
    w3 = np.ascontiguousarray(np.asarray(w3, dtype=np.float32))

    # Host routing: unique tokens per expert, with both top-k gate weights of
    # a token merged (a token picking the same expert twice gets the summed
    # gate).
    tok_lists, gate_lists = [], []
    for e in range(E):
        m = idx == e
        sel = np.nonzero(m.any(axis=1))[0]
        tok_lists.append(sel)
        gate_lists.append((ew * m).sum(axis=1)[sel].astype(np.float32))

    weight_maps = _pack_weights(w1, w2, w3)

    n_pass = max(1, math.ceil(max(len(s) for s in tok_lists) / CAP))
    out = np.zeros((T, D), dtype=np.float32)
    trace = bool(os.environ.get("BASS_TRACE"))
    for p in range(n_pass):
        chunks = [s[p * CAP:(p + 1) * CAP] for s in tok_lists]
        gates = [g[p * CAP:(p + 1) * CAP] for g in gate_lists]
        ncols = tuple(max(8, -(-len(s) // 8) * 8) for s in chunks)
        if ncols not in _NCS:
            _NCS[ncols] = _build_nc(ncols)
        nc_mod = _NCS[ncols]

        off = [0]
        for n in ncols:
            off.append(off[-1] + n)
        SN = off[-1]
        xt = np.zeros((128, DT, SN), dtype=BF16_NP)
        g_pad = np.zeros((128, 4 * E), dtype=np.float32)
        for e in range(E):
            sel = chunks[e]
            if len(sel):
                xe = np.zeros((D, ncols[e]), dtype=np.float32)
                xe[:, :len(sel)] = x[sel].T
                xt[:, :, off[e]:off[e + 1]] = (
                    xe.astype(BF16_NP).reshape(DT, 128, ncols[e]).transpose(1, 0, 2)
                )
                gp = np.zeros((512,), dtype=np.float32)
                gp[:len(sel)] = gates[e]
                g_pad[:, 4 * e:4 * e + 4] = gp.reshape(4, 128).T
        in_maps = [
            {"xt": xt, "g": g_pad, **weight_maps[c]} for c in range(N_CORES)
        ]
        # Rare transient NRT_EXEC_UNIT_UNRECOVERABLE errors have been
        # observed on the first execution of a fresh NEFF; a straight retry
        # recovers.
        last_exc = None
        for attempt in range(3):
            try:
                LAST_RESULTS = run_bass_kernel_spmd(
                    nc_mod, in_maps, core_ids=list(range(N_CORES)),
                    trace=trace and attempt == 0,
                )
                break
            except Exception as exc:  # noqa: BLE001
                last_exc = exc
                time.sleep(3)
        else:
            raise last_exc
        for c in range(N_CORES):
            yc = LAST_RESULTS.results[c]["y"]
            for e in range(E):
                sel = chunks[e]
                if len(sel):
                    out[sel] += yc[off[e]:off[e] + len(sel)].astype(np.float32)
    return out
